# revision 7
# baseline (speedup 1.0000x reference)
"""Mamba2-style chunked SSD scan on 8 Trainium2 NeuronCores.

Full-input contract: kernel(X, A, B, C, initial_states) -> Y, with
  X: (b, s, h, p) f32   A: (b, s, h) f32   B, C: (b, s, h, n) f32
  initial_states: (b, 1, h, p, n) f32      Y: (b, s, h, p) f32

Sharding: heads across the 8 cores (h % 8 == 0); every core runs an
identical program over its own (b, h/8) slice -- no collectives.

Math (block length L=128; c = s/L chunks per (b,h) stream; cum = within-
chunk inclusive cumsum of A):  with host-prescaled
    Bt[s,n] = B[s,n] * exp(-cum_s)      (bf16)
    Ct[i,n] = C[i,n] * exp(+cum_i)      (bf16)
the chunk output and state recurrence collapse to plain matmuls:
    Y[i,p] = sum_{s<=i} (Bt Ct^T)[s,i] X[s,p]  +  (Ct R)[i,p]
    R     <- exp(tot) * ( R + Bt^T X )
so the device does, per chunk (all 8 (b,h) bodies batched side by side):
    G   = Bt^T-slices @ Ct^T-slices      8 matmuls -> one PSUM tile
    M   = G * mask(s<=i)                 1 DVE op (bf16 out)
    Y   = M^T @ X + Ct @ R_prev          16 matmuls -> one PSUM tile
    S   = Bt^T @ X                       8 matmuls -> one PSUM tile
    R  += S; R *= exp(tot); rb = bf16(R) 1 DVE + 1 GpSimd + 1 ACT op
Host supplies Bt/Ct already transposed (n-major) so the kernel needs no
on-device transposes, and chunk PAIRS share single wide DMAs.
"""

import os
from functools import lru_cache

import ml_dtypes
import numpy as np

L = 128  # chunk/block length (our choice; any block size is math-equivalent)
N_CORES = 8

_f32 = np.float32
_bf16 = ml_dtypes.bfloat16

BODY_F = 132  # per-body column block in BCX: [X(64) | Bt(64) | aux(4)]


def _maybe_enable_tracing():
    """Optional NTFF profiling (BASS_KERNEL_TRACE=1). Never required."""
    if not os.environ.get("BASS_KERNEL_TRACE"):
        return False
    try:
        import sys
        import types

        if "antenv.axon_hooks" not in sys.modules:
            mod = types.ModuleType("antenv.axon_hooks")
            mod._hook = None
            mod.set_axon_ntff_profile_hook = lambda h: setattr(mod, "_hook", h)
            mod.get_axon_ntff_profile_hook = lambda: mod._hook
            sys.modules["antenv.axon_hooks"] = mod
            from trn_agent_boot.trn_boot import _ntff_profile_via_ctypes

            hook = _ntff_profile_via_ctypes("/opt/axon/libaxon_pjrt.so")
            if hook is None:
                return False
            mod.set_axon_ntff_profile_hook(hook)
            import concourse.bass_utils as bu

            bu.upload_artifacts = lambda tmpdir: f"file://{tmpdir}"
        return True
    except Exception:
        return False


@lru_cache(maxsize=4)
def _build_program(b, s, hpc, p, n):
    """Build + compile the per-core Bass program.

    Per-core DRAM tensors (c2 = chunk pairs, nbh = b*hpc bodies, F = nbh*BODY_F):
      BCX  (c2, L, 2*F)        bf16  [X | Bt | aux] per body, chunk pair in free
      TBT  (c2, n, 2*nbh*L)    bf16  Bt^T (n-major)
      TCT  (c2, n, 2*nbh*L)    bf16  Ct^T (n-major)
      INIT (n, nbh*p)          f32   initial states
      MASKX(L, nbh*L)          bf16  mask[s, i] = (s <= i), tiled per body
      Y    (c2, L, 2*nbh*p)    f32
    """
    import concourse.bacc as bacc
    import concourse.mybir as mybir
    import concourse.tile as tile

    dt = mybir.dt
    assert s % (2 * L) == 0 and p == 64 and n == 64
    c = s // L
    c2 = c // 2
    nbh = b * hpc
    F = nbh * BODY_F
    FB = nbh * L  # tbt/tct per-chunk free size
    FP = nbh * p  # y/s per-chunk free size

    nc = bacc.Bacc("TRN2", target_bir_lowering=False, debug=False)

    bcx_d = nc.dram_tensor("BCX", [c2, L, 2 * F], dt.bfloat16, kind="ExternalInput").ap()
    tbt_d = nc.dram_tensor("TBT", [c2, n, 2 * FB], dt.bfloat16, kind="ExternalInput").ap()
    tct_d = nc.dram_tensor("TCT", [c2, n, 2 * FB], dt.bfloat16, kind="ExternalInput").ap()
    init_d = nc.dram_tensor("INIT", [n, FP], dt.float32, kind="ExternalInput").ap()
    mask_d = nc.dram_tensor("MASKX", [L, FB], dt.bfloat16, kind="ExternalInput").ap()
    y_d = nc.dram_tensor("Y", [c2, p, 2 * FB], dt.float32, kind="ExternalOutput").ap()

    with tile.TileContext(nc) as tc:
        with (
            tc.tile_pool(name="const", bufs=1) as cpool,
            tc.tile_pool(name="state", bufs=1) as rpool,
            tc.tile_pool(name="rb", bufs=2) as rbpool,
            tc.tile_pool(name="io", bufs=3) as iopool,
            tc.tile_pool(name="tp", bufs=3) as tpool,
            tc.tile_pool(name="work", bufs=3) as wpool,
            tc.tile_pool(name="out", bufs=3) as opool,
            tc.tile_pool(name="psG", bufs=2, space="PSUM") as psG,
            tc.tile_pool(name="psYS", bufs=2, space="PSUM") as psYS,
        ):
            maskx = cpool.tile([L, FB], dt.bfloat16, tag="maskx")
            nc.sync.dma_start(maskx[:], mask_d[:])

            r_big = rpool.tile([n, FP], dt.float32, tag="R")
            nc.sync.dma_start(r_big[:], init_d[:])
            rb_prev = rbpool.tile([n, FP], dt.bfloat16, tag="rb")
            nc.scalar.copy(rb_prev[:], r_big[:])

            for cp in range(c2):
                bcx2 = iopool.tile([L, 2 * F], dt.bfloat16, tag="bcx")
                nc.sync.dma_start(bcx2[:], bcx_d[cp])
                tbt2 = tpool.tile([n, 2 * FB], dt.bfloat16, tag="tbt")
                nc.sync.dma_start(tbt2[:], tbt_d[cp])
                tct2 = tpool.tile([n, 2 * FB], dt.bfloat16, tag="tct")
                nc.sync.dma_start(tct2[:], tct_d[cp])
                yout2 = opool.tile([2 * p, 2 * FB], dt.float32, tag="yout")

                bcx_f32 = bcx2[:].bitcast(dt.float32)  # (L, F)

                for j in range(2):
                    gps = psG.tile([L, FB], dt.float32, tag="gps")
                    for i in range(nbh):
                        tb_i = tbt2[:, j * FB + i * L : j * FB + (i + 1) * L]
                        tc_i = tct2[:, j * FB + i * L : j * FB + (i + 1) * L]
                        nc.tensor.matmul(
                            gps[:, i * L : (i + 1) * L], tb_i, tc_i,
                            start=True, stop=True,
                        )

                    m1 = wpool.tile([L, FB], dt.bfloat16, tag="m1")
                    nc.vector.tensor_mul(m1[:], gps[:], maskx[:])

                    ysps = psYS.tile([2 * p, FB], dt.float32, tag="ysps")
                    sps = ysps[0:n, 0:FP]
                    yps = ysps[p : 2 * p, :]
                    for i in range(nbh):
                        x_i = bcx2[:, j * F + i * BODY_F : j * F + i * BODY_F + p]
                        bt_i = bcx2[
                            :, j * F + i * BODY_F + p : j * F + i * BODY_F + 2 * p
                        ]
                        tc_i = tct2[:, j * FB + i * L : j * FB + (i + 1) * L]
                        ys = yps[:, i * L : (i + 1) * L]
                        nc.tensor.matmul(
                            ys, x_i, m1[:, i * L : (i + 1) * L],
                            start=True, stop=False, tile_position=(0, p),
                        )
                        nc.tensor.matmul(
                            ys, rb_prev[:, i * p : (i + 1) * p], tc_i,
                            start=False, stop=True, tile_position=(0, p),
                        )
                        nc.tensor.matmul(
                            sps[:, i * p : (i + 1) * p], bt_i, x_i,
                            start=True, stop=True,
                        )

                    nc.scalar.copy(yout2[p : 2 * p, j * FB : (j + 1) * FB], yps[:])

                    # R <- exp(tot) * (R + S); rb = bf16(R)
                    nc.vector.tensor_add(r_big[:], r_big[:], sps[:])
                    d_bc = (
                        bcx_f32[0:n, j * F // 2 : (j + 1) * F // 2]
                        .rearrange("q (i f) -> q i f", f=BODY_F // 2)[:, :, p : p + 1]
                        .broadcast_to((n, nbh, p))
                    )
                    r_3d = r_big[:].rearrange("q (i f) -> q i f", f=p)
                    nc.gpsimd.tensor_mul(r_3d, r_3d, d_bc)
                    rb = rbpool.tile([n, FP], dt.bfloat16, tag="rb")
                    nc.scalar.copy(rb[:], r_big[:])
                    rb_prev = rb

                nc.sync.dma_start(y_d[cp], yout2[p : 2 * p, :])

    nc.compile()
    return nc


def _host_prep(X, A, B, C, initial_states, hpc):
    """Build the packed/prescaled per-core input arrays."""
    b, s, h, p = X.shape
    n = B.shape[-1]
    c = s // L
    c2 = c // 2

    # within-chunk inclusive cumsum of A: (b, h, c, L)
    Ar = np.ascontiguousarray(A.reshape(b, c, L, h).transpose(0, 3, 1, 2))
    cum = np.cumsum(Ar, axis=-1, dtype=_f32)
    # align to (b, c, L, h) for broadcasting against B/C/X reshapes
    e_neg = np.exp(-cum).transpose(0, 2, 3, 1)[..., None]  # (b, c, L, h, 1)
    e_pos = np.exp(cum).transpose(0, 2, 3, 1)[..., None]
    d = np.exp(cum[..., -1])  # (b, h, c)

    Bt = (B.reshape(b, c, L, h, n) * e_neg).astype(_bf16)
    Ct = (C.reshape(b, c, L, h, n) * e_pos).astype(_bf16)

    # BCX: (c2, L, 2, b, h, BODY_F) -> flatten
    full = np.zeros((b, c, L, h, BODY_F), dtype=_bf16)
    full[..., 0:p] = X.reshape(b, c, L, h, p).astype(_bf16)
    full[..., p : p + n] = Bt
    daux = np.zeros((b, c, L, h, 2), dtype=_f32)
    daux[..., 0] = d.transpose(0, 2, 1)[:, :, None, :]  # (b, c, 1, h) broadcast over L
    full[..., p + n : p + n + 4] = daux.view(np.uint16).view(_bf16)
    bcx = np.ascontiguousarray(
        full.reshape(b, c2, 2, L, h, BODY_F).transpose(1, 3, 2, 0, 4, 5)
    )  # (c2, L, 2, b, h, BODY_F)

    # TBT/TCT: (c2, n, 2, b, h, L)
    tbt = np.ascontiguousarray(
        Bt.reshape(b, c2, 2, L, h, n).transpose(1, 5, 2, 0, 4, 3)
    )
    tct = np.ascontiguousarray(
        Ct.reshape(b, c2, 2, L, h, n).transpose(1, 5, 2, 0, 4, 3)
    )

    # INIT: (b, h, p, n) -> (n, b, h, p)
    init_t = np.ascontiguousarray(
        initial_states[:, 0].transpose(3, 0, 1, 2)
    ).astype(_f32)

    return bcx, tbt, tct, init_t


def kernel(X, A, B, C, initial_states):
    from concourse.bass_utils import run_bass_kernel_spmd

    X = np.asarray(X)
    A = np.asarray(A)
    B = np.asarray(B)
    C = np.asarray(C)
    initial_states = np.asarray(initial_states)

    b, s, h, p = X.shape
    n = B.shape[-1]
    assert h % N_CORES == 0, f"need h % {N_CORES} == 0, got h={h}"
    hpc = h // N_CORES
    c = s // L
    c2 = c // 2
    nbh = b * hpc

    bcx, tbt, tct, init_t = _host_prep(X, A, B, C, initial_states, hpc)

    mask = np.triu(np.ones((L, L), dtype=_f32)).astype(_bf16)
    maskx = np.ascontiguousarray(np.broadcast_to(mask[:, None, :], (L, nbh, L)))

    nc = _build_program(b, s, hpc, p, n)

    in_maps = []
    for k in range(N_CORES):
        hs = slice(k * hpc, (k + 1) * hpc)
        in_maps.append(
            {
                "BCX": np.ascontiguousarray(bcx[:, :, :, :, hs]).reshape(
                    c2, L, 2 * nbh * BODY_F
                ),
                "TBT": np.ascontiguousarray(tbt[:, :, :, :, hs]).reshape(
                    c2, n, 2 * nbh * L
                ),
                "TCT": np.ascontiguousarray(tct[:, :, :, :, hs]).reshape(
                    c2, n, 2 * nbh * L
                ),
                "INIT": np.ascontiguousarray(init_t[:, :, hs]).reshape(n, nbh * p),
                "MASKX": maskx.reshape(L, nbh * L),
            }
        )

    trace = _maybe_enable_tracing()
    kw = {}
    if trace:
        kw = dict(trace=True, tmpdir=os.environ.get("BASS_KERNEL_TRACE_DIR") or None)
    res = run_bass_kernel_spmd(nc, in_maps, list(range(N_CORES)), **kw)
    if trace and res.exec_time_ns is not None:
        print(f"HW exec time: {res.exec_time_ns} ns")

    # Y per core: (c2, L, 2, b, hpc, p) -> (b, s, hpc, p); concat heads
    ys = []
    for k in range(N_CORES):
        yk = res.results[k]["Y"].reshape(c2, p, 2, b, hpc, L)
        ys.append(np.transpose(yk, (3, 0, 2, 5, 4, 1)))  # (b, c2, 2, L, hpc, p)
    Y = np.concatenate(ys, axis=4).reshape(b, s, h, p)
    return np.ascontiguousarray(Y).astype(_f32)
